# revision 7
# baseline (speedup 1.0000x reference)
"""CenterLoss Trainium2 kernel.

Computes, given features (512, 2048) f32, labels (512,) int, centers
(10000, 2048) f32:
  - center_loss = mean((features - centers[labels])**2)
  - new_centers = sequential per-sample EMA update of centers:
        for i in batch order: c[l_i] <- c[l_i] + 0.5 * (f_i - c[l_i])

The sequential EMA has a closed form per label l with occurrences
i_1 < ... < i_k:
    new_c[l] = 0.5**k * c[l] + sum_j 0.5**(k-j+1) * f[i_j]
so the scan becomes: per-sample weights w_i (computed on host from the
labels alone), a same-label weighted segment-sum U = A @ F (A[i,j] =
[l_i==l_j] * w_j, host-built), a per-label decay, and a scatter of
identical duplicate rows.

Sharding: tensor-parallel along feature_dim across 8 cores (256 cols
each). Each core: bulk-copies its (10000, 256) centers slice to the
output (memory-bound part), gathers the 512 label rows (indirect DMA),
computes its slice of the loss + updated rows, scatters the updated
rows over the copy. The scalar loss partials are summed on host.
"""

import sys

sys.path.insert(0, "/opt/trn_rl_repo")

import numpy as np

NUM_CLASSES = 10000
FEATURE_DIM = 2048
BATCH = 512
ALPHA = 0.5
NCORES = 8
DS = FEATURE_DIM // NCORES  # 256 per-core feature slice
P = 128  # SBUF partitions
T = BATCH // P  # 4 batch tiles

_cached_nc = None


def _build_program():
    import concourse.bass as bass
    import concourse.tile as tile
    from concourse import bacc, mybir

    nc = bacc.Bacc("TRN2", target_bir_lowering=False, debug=False,
                   num_devices=NCORES)
    f32 = mybir.dt.float32

    centers = nc.dram_tensor("centers", [NUM_CLASSES, DS], f32,
                             kind="ExternalInput").ap()
    feat = nc.dram_tensor("feat", [P, T * DS], f32, kind="ExternalInput").ap()
    at = nc.dram_tensor("at", [P, T * BATCH], f32, kind="ExternalInput").ap()
    idx = nc.dram_tensor("idx", [P, T], mybir.dt.int32,
                         kind="ExternalInput").ap()
    decay = nc.dram_tensor("decay", [P, T], f32, kind="ExternalInput").ap()
    out_c = nc.dram_tensor("out_centers", [NUM_CLASSES, DS], f32,
                           kind="ExternalOutput").ap()
    loss = nc.dram_tensor("loss_part", [P, 1], f32, kind="ExternalOutput").ap()

    with tile.TileContext(nc) as tc:
        with (
            tc.tile_pool(name="sbuf", bufs=1) as sp,
            tc.tile_pool(name="psum", bufs=1, space="PSUM") as pp,
        ):
            # Small loads on the ACT HWDGE ring so they don't queue behind
            # the bulk copy (which goes on the SP ring).
            f_sb = sp.tile([P, T * DS], f32)
            nc.scalar.dma_start(out=f_sb[:], in_=feat[:])
            at_sb = sp.tile([P, T * BATCH], f32)
            nc.scalar.dma_start(out=at_sb[:], in_=at[:])
            idx_sb = sp.tile([P, T], mybir.dt.int32)
            nc.scalar.dma_start(out=idx_sb[:], in_=idx[:])
            dec_sb = sp.tile([P, T], f32)
            nc.scalar.dma_start(out=dec_sb[:], in_=decay[:])

            # Gather G[p, t*DS:(t+1)*DS] = centers[labels[t*P+p]] (SWDGE).
            # HW indirect DMA consumes one index per SBUF partition, so one
            # transfer per batch tile of 128 rows.
            g_sb = sp.tile([P, T * DS], f32)
            for t in range(T):
                nc.gpsimd.indirect_dma_start(
                    out=g_sb[:, t * DS:(t + 1) * DS],
                    out_offset=None,
                    in_=centers[:],
                    in_offset=bass.IndirectOffsetOnAxis(
                        ap=idx_sb[:, t:t + 1], axis=0),
                )

            # Loss slice: sum over (F - G)^2, accumulated per partition.
            diff = sp.tile([P, T * DS], f32)
            nc.vector.tensor_sub(diff[:], f_sb[:], g_sb[:])
            sq = sp.tile([P, T * DS], f32)
            loss_col = sp.tile([P, 1], f32)
            nc.scalar.activation(
                out=sq[:], in_=diff[:],
                func=mybir.ActivationFunctionType.Square,
                accum_out=loss_col[:],
            )
            nc.scalar.dma_start(out=loss[:], in_=loss_col[:])

            # U = A @ F (PE), then v = decay * G + U (DVE), per batch tile.
            v_sb = sp.tile([P, T * DS], f32)
            for t in range(T):
                u_ps = pp.tile([P, DS], f32, space="PSUM")
                for k in range(T):
                    nc.tensor.matmul(
                        out=u_ps[:],
                        lhsT=at_sb[:, k * BATCH + t * P:k * BATCH + (t + 1) * P],
                        rhs=f_sb[:, k * DS:(k + 1) * DS],
                        start=(k == 0),
                        stop=(k == T - 1),
                    )
                nc.vector.scalar_tensor_tensor(
                    out=v_sb[:, t * DS:(t + 1) * DS],
                    in0=g_sb[:, t * DS:(t + 1) * DS],
                    scalar=dec_sb[:, t:t + 1],
                    in1=u_ps[:],
                    op0=mybir.AluOpType.mult,
                    op1=mybir.AluOpType.add,
                )

            # Bulk copy centers -> out_centers as ONE flat DMA on the
            # otherwise-idle SP HWDGE ring. Emitted late so no compute
            # wait counts its completion increment (the 8 DMAHW sem
            # lanes use cumulative counts; sharing a lane with the slow
            # copy would falsely gate compute on it). It still
            # dispatches immediately: the Sync engine has nothing else.
            flat_in = centers.rearrange("a b -> (a b)")
            flat_out = out_c.rearrange("a b -> (a b)")
            nc.sync.dma_start(out=flat_out[:], in_=flat_in[:])

            # Scatter updated rows over the bulk copy. Duplicate labels
            # scatter identical rows, so write order doesn't matter.
            for t in range(T):
                nc.gpsimd.indirect_dma_start(
                    out=out_c[:],
                    out_offset=bass.IndirectOffsetOnAxis(
                        ap=idx_sb[:, t:t + 1], axis=0),
                    in_=v_sb[:, t * DS:(t + 1) * DS],
                    in_offset=None,
                )
    nc.compile()
    return nc


def _get_program():
    global _cached_nc
    if _cached_nc is None:
        _cached_nc = _build_program()
    return _cached_nc


def _host_prep(features, labels, centers):
    """Build per-core input maps. Returns (in_maps, denom)."""
    features = np.ascontiguousarray(np.asarray(features, dtype=np.float32))
    centers = np.asarray(centers, dtype=np.float32)
    labels = np.asarray(labels).astype(np.int64)

    # Per-sample EMA weights from the label sequence alone.
    # occurrence index o_i (0-based) and total count k per label:
    #   w_i = 0.5**(k - o_i), decay_i = 0.5**k
    counts = {}
    occ = np.empty(BATCH, dtype=np.int64)
    for i, l in enumerate(labels):
        c = counts.get(int(l), 0)
        occ[i] = c
        counts[int(l)] = c + 1
    k = np.array([counts[int(l)] for l in labels], dtype=np.int64)
    w = (0.5 ** (k - occ)).astype(np.float32)
    dec = (0.5 ** k).astype(np.float32)

    # A[i, j] = [l_i == l_j] * w_j ; shipped as AT in matmul lhsT layout.
    A = (labels[:, None] == labels[None, :]).astype(np.float32) * w[None, :]
    AT = np.ascontiguousarray(A.T)
    at_sb = np.ascontiguousarray(
        AT.reshape(T, P, BATCH).transpose(1, 0, 2).reshape(P, T * BATCH))

    idx_np = np.ascontiguousarray(labels.reshape(T, P).T).astype(np.int32)
    dec_np = np.ascontiguousarray(dec.reshape(T, P).T).astype(np.float32)

    in_maps = []
    for c in range(NCORES):
        fc = features[:, c * DS:(c + 1) * DS]
        f_sb = np.ascontiguousarray(
            fc.reshape(T, P, DS).transpose(1, 0, 2).reshape(P, T * DS))
        cc = np.ascontiguousarray(centers[:, c * DS:(c + 1) * DS])
        in_maps.append({
            "centers": cc,
            "feat": f_sb,
            "at": at_sb,
            "idx": idx_np,
            "decay": dec_np,
        })
    return in_maps


def run(features, labels, centers, trace=False, **trace_kwargs):
    """Run the device kernel; returns (loss, new_centers, BassKernelResults)."""
    from concourse.bass_utils import run_bass_kernel_spmd

    nc = _get_program()
    in_maps = _host_prep(features, labels, centers)
    res = run_bass_kernel_spmd(nc, in_maps, list(range(NCORES)), trace=trace,
                               **trace_kwargs)
    new_centers = np.concatenate(
        [res.results[c]["out_centers"] for c in range(NCORES)], axis=1)
    sumsq = np.sum([res.results[c]["loss_part"].astype(np.float64).sum()
                    for c in range(NCORES)])
    loss = np.float32(sumsq / (BATCH * FEATURE_DIM))
    return loss, new_centers, res


def kernel(features, labels, centers):
    loss, new_centers, _ = run(features, labels, centers)
    return loss, new_centers
